# revision 2
# baseline (speedup 1.0000x reference)
"""Grouped SwiGLU MoE (16 experts, top-2, 8192x1024 tokens, d_ff 2816) on 8 TRN2 cores.

Expert-parallel, 2 experts per core. Host does the integer routing (sort
tokens by expert), pairs heavy experts with light ones so the two per-core
segment lengths (L0 >= L1) are data-tight instead of a global worst-case cap,
and pre-casts activations/weights to bf16 (PE runs bf16 at full rate; rel-err
budget 2e-2 >> bf16 noise ~2e-3). The routing weight is folded into a second,
pre-scaled copy of x that feeds the w2 (value) path - the w3 GEMM is linear,
so the final per-token multiply disappears and the output can stream to HBM
during the last weight group. Everything device-side is Bass/Tile via
run_bass_kernel_spmd on cores 0-7.

Per-core compute: y^T = w3^T @ (silu(w1^T x^T) * (w2^T (w.x)^T)), all in
feature-major layout so no on-chip transposes are needed.
"""

import numpy as np

N_EXPERTS, D_MODEL, D_FF = 16, 1024, 2816
N_TOKENS, TOP_K = 8192, 2
N_CORES = 8
E_LOCAL = N_EXPERTS // N_CORES  # 2 expert slots per core
DK = D_MODEL // 128             # 8 contraction tiles for x
FK = D_FF // 128                # 22 f tiles
F_GROUP = 4                     # f-tiles per streamed weight group

# ----------------------------------------------------------------- host utils


def _to_bf16(x: np.ndarray) -> np.ndarray:
    import ml_dtypes

    return np.ascontiguousarray(x, dtype=np.float32).astype(ml_dtypes.bfloat16)


def _slice_plan(L: int) -> list[tuple[int, int]]:
    """Split L into (start, width) pieces: greedy 512s, arbitrary tail."""
    out, s = [], 0
    while s < L:
        w = min(512, L - s)
        out.append((s, w))
        s += w
    return out


# ------------------------------------------------------- walrus wait-split fix


def _split_excess_waits(nc):
    """This walrus build encodes at most ONE sync wait per instruction; Tile
    can attach several (first matmul of a group, kernel-tail drain). Hoist the
    excess into standalone InstEventSemaphore (the shape wait_ge emits)."""
    import bass_rust
    import concourse.mybir as mybir

    n = 0
    for fn in nc.m.functions:
        for blk in fn.blocks:
            out, changed = [], False
            for inst in blk.instructions:
                si = inst.sync_info
                if si is not None and si.on_wait is not None and len(si.on_wait) > 1:
                    waits = list(si.on_wait)
                    for w in waits[:-1]:
                        ev = mybir.InstEventSemaphore(name=f"I-wsplit-{n}", ins=[], outs=[])
                        n += 1
                        ev.engine = inst.engine
                        ev.sync_info = bass_rust.SyncInfo(on_wait=[w], on_update=[])
                        out.append(ev)
                    inst.sync_info = bass_rust.SyncInfo(
                        on_wait=waits[-1:], on_update=list(si.on_update or [])
                    )
                    changed = True
                out.append(inst)
            if changed:
                blk.instructions = out
    return n


# ------------------------------------------------------------- device program


def _build(L0: int, L1: int):
    import concourse.bass as bass
    import concourse.tile as tile
    import concourse.mybir as mybir

    f32 = mybir.dt.float32
    bf16 = mybir.dt.bfloat16
    Ls = (L0, L1)

    groups = []
    f0 = 0
    while f0 < FK:
        glen = min(F_GROUP, FK - f0)
        groups.append((f0, glen))
        f0 += glen
    n_groups = len(groups)

    nc = bass.Bass()
    xg_d = [nc.dram_tensor(f"xg{e}", [DK, 128, Ls[e]], bf16, kind="ExternalInput")
            for e in range(E_LOCAL)]
    xv_d = [nc.dram_tensor(f"xv{e}", [DK, 128, Ls[e]], bf16, kind="ExternalInput")
            for e in range(E_LOCAL)]
    w1t = nc.dram_tensor("w1t", [E_LOCAL, DK, 128, D_FF], bf16, kind="ExternalInput")
    w2t = nc.dram_tensor("w2t", [E_LOCAL, DK, 128, D_FF], bf16, kind="ExternalInput")
    w3t = nc.dram_tensor("w3t", [E_LOCAL, FK, 128, D_MODEL], bf16, kind="ExternalInput")
    yt_d = [nc.dram_tensor(f"yt{e}", [DK, 128, Ls[e]], f32, kind="ExternalOutput")
            for e in range(E_LOCAL)]

    with tile.TileContext(nc) as tc:
        with (
            tc.tile_pool(name="xts", bufs=1) as p_x,
            tc.tile_pool(name="w12", bufs=2) as p_w12,
            tc.tile_pool(name="w3", bufs=2) as p_w3,
            tc.tile_pool(name="hs", bufs=2 * F_GROUP) as p_hs,
            tc.tile_pool(name="sil", bufs=3) as p_sil,
            tc.tile_pool(name="yacc", bufs=1) as p_y,
            tc.tile_pool(name="gv", bufs=4, space="PSUM") as p_gv,
            tc.tile_pool(name="py", bufs=2, space="PSUM") as p_py,
        ):
            # stage all x up-front, sliced so the first GEMM can start early;
            # distinct tags per expert slot = independent buffers, no reload
            # stall at the expert boundary.
            xg_t, xv_t = [], []
            for e in range(E_LOCAL):
                L = Ls[e]
                xg = p_x.tile([128, DK, L], bf16, tag=f"xg{e}")
                xv = p_x.tile([128, DK, L], bf16, tag=f"xv{e}")
                xg_t.append(xg)
                xv_t.append(xv)
            for s0, w in _slice_plan(Ls[0]) + [(None, None)]:
                # interleave: expert-0 slices first (critical path), then
                # expert-1 wholesale
                if s0 is None:
                    break
                for half in range(2):
                    hs_, he_ = half * (DK // 2), (half + 1) * (DK // 2)
                    nc.sync.dma_start(
                        out=xg_t[0][:, hs_:he_, s0:s0 + w],
                        in_=xg_d[0][hs_:he_, :, s0:s0 + w].rearrange("a p t -> p a t"),
                    )
                    nc.sync.dma_start(
                        out=xv_t[0][:, hs_:he_, s0:s0 + w],
                        in_=xv_d[0][hs_:he_, :, s0:s0 + w].rearrange("a p t -> p a t"),
                    )
            for e in range(1, E_LOCAL):
                for half in range(2):
                    hs_, he_ = half * (DK // 2), (half + 1) * (DK // 2)
                    nc.sync.dma_start(
                        out=xg_t[e][:, hs_:he_, :],
                        in_=xg_d[e][hs_:he_].rearrange("a p t -> p a t"),
                    )
                    nc.sync.dma_start(
                        out=xv_t[e][:, hs_:he_, :],
                        in_=xv_d[e][hs_:he_].rearrange("a p t -> p a t"),
                    )

            for e in range(E_LOCAL):
                L = Ls[e]
                slices = _slice_plan(L)
                xg, xv = xg_t[e], xv_t[e]
                y_acc = p_y.tile([128, DK, Ls[0]], f32, tag="yacc")

                for gi, (f0, glen) in enumerate(groups):
                    last_g = gi == n_groups - 1
                    fw = glen * 128
                    w1r = p_w12.tile([128, DK, F_GROUP * 128], bf16, tag="w1r")
                    w2r = p_w12.tile([128, DK, F_GROUP * 128], bf16, tag="w2r")
                    # w1t[e] is [dk, p, f]; SBUF wants [p, dk, f-slice]
                    for half in range(2):
                        hs_, he_ = half * (DK // 2), (half + 1) * (DK // 2)
                        nc.sync.dma_start(
                            out=w1r[:, hs_:he_, :fw],
                            in_=w1t[e, hs_:he_, :, f0 * 128:f0 * 128 + fw].rearrange(
                                "a p f -> p a f"
                            ),
                        )
                        nc.sync.dma_start(
                            out=w2r[:, hs_:he_, :fw],
                            in_=w2t[e, hs_:he_, :, f0 * 128:f0 * 128 + fw].rearrange(
                                "a p f -> p a f"
                            ),
                        )
                    w3r = p_w3.tile([128, F_GROUP, DK, 128], bf16, tag="w3r")
                    for half in range(2):
                        hs_ = half * (glen // 2) if glen > 1 else 0
                        he_ = (half + 1) * (glen // 2) if glen > 1 else glen
                        if hs_ == he_:
                            continue
                        nc.sync.dma_start(
                            out=w3r[:, hs_:he_, :, :],
                            in_=w3t[e, f0 + hs_:f0 + he_].rearrange(
                                "a p (b d) -> p a b d", b=DK
                            ),
                        )

                    for s0, w in slices:
                        hs_tiles = []
                        for fl in range(glen):
                            pg = p_gv.tile([128, 512], f32, tag="gv")
                            for dk in range(DK):
                                nc.tensor.matmul(
                                    pg[:, :w],
                                    w1r[:, dk, fl * 128:(fl + 1) * 128],
                                    xg[:, dk, s0:s0 + w],
                                    start=(dk == 0),
                                    stop=(dk == DK - 1),
                                )
                            pv = p_gv.tile([128, 512], f32, tag="gv")
                            for dk in range(DK):
                                nc.tensor.matmul(
                                    pv[:, :w],
                                    w2r[:, dk, fl * 128:(fl + 1) * 128],
                                    xv[:, dk, s0:s0 + w],
                                    start=(dk == 0),
                                    stop=(dk == DK - 1),
                                )
                            sil = p_sil.tile([128, 512], f32, tag="sil")
                            nc.scalar.activation(
                                sil[:, :w], pg[:, :w], mybir.ActivationFunctionType.Silu
                            )
                            hst = p_hs.tile([128, 512], bf16, tag="hs")
                            nc.vector.tensor_mul(hst[:, :w], sil[:, :w], pv[:, :w])
                            hs_tiles.append(hst)

                        for di in range(DK):
                            py = p_py.tile([128, 512], f32, tag="py")
                            for fl in range(glen):
                                nc.tensor.matmul(
                                    py[:, :w],
                                    w3r[:, fl, di, :],
                                    hs_tiles[fl][:, :w],
                                    start=(fl == 0),
                                    stop=(fl == glen - 1),
                                )
                            if gi == 0:
                                nc.vector.tensor_copy(y_acc[:, di, s0:s0 + w], py[:, :w])
                            else:
                                nc.vector.tensor_add(
                                    y_acc[:, di, s0:s0 + w],
                                    y_acc[:, di, s0:s0 + w],
                                    py[:, :w],
                                )
                            if last_g:
                                nc.sync.dma_start(
                                    out=yt_d[e][di, :, s0:s0 + w],
                                    in_=y_acc[:, di, s0:s0 + w],
                                )

    _split_excess_waits(nc)
    return nc


_BUILD_CACHE: dict[tuple, object] = {}


def _get_nc(L0: int, L1: int):
    key = (L0, L1)
    if key not in _BUILD_CACHE:
        _BUILD_CACHE[key] = _build(L0, L1)
    return _BUILD_CACHE[key]


# -------------------------------------------------------------------- kernel


def prepare(x, expert_indices, expert_weights, w1, w2, w3):
    """Host routing + sharding. Returns (nc, in_maps, meta)."""
    x = np.asarray(x, dtype=np.float32)
    ei = np.asarray(expert_indices).reshape(-1)
    ew = np.asarray(expert_weights).reshape(-1).astype(np.float32)

    # ---- integer routing on host (replicated bookkeeping)
    order = np.argsort(ei, kind="stable")
    tok_sorted = (np.repeat(np.arange(N_TOKENS, dtype=np.int64), TOP_K))[order]
    w_sorted = ew[order]
    counts = np.bincount(ei, minlength=N_EXPERTS)
    seg = np.concatenate(([0], np.cumsum(counts)))

    # ---- pair heavy experts with light ones; slot 0 takes the bigger one
    by_size = np.argsort(counts, kind="stable")[::-1]  # expert ids, desc count
    slot_expert = np.zeros((N_CORES, E_LOCAL), dtype=np.int64)
    for c in range(N_CORES):
        slot_expert[c, 0] = by_size[c]
        slot_expert[c, 1] = by_size[N_EXPERTS - 1 - c]
    L0 = int(max(256, -(-int(counts[slot_expert[:, 0]].max()) // 4) * 4))
    L1 = int(max(256, -(-int(counts[slot_expert[:, 1]].max()) // 4) * 4))
    Ls = (L0, L1)

    # ---- bf16 pre-casting
    xb = _to_bf16(x)
    w1b = _to_bf16(np.asarray(w1))
    w2b = _to_bf16(np.asarray(w2))
    w3b = _to_bf16(np.asarray(w3))
    import ml_dtypes

    # ---- shard per core
    in_maps = []
    for c in range(N_CORES):
        m = {}
        w1_c = np.empty((E_LOCAL, DK, 128, D_FF), dtype=ml_dtypes.bfloat16)
        w2_c = np.empty((E_LOCAL, DK, 128, D_FF), dtype=ml_dtypes.bfloat16)
        w3_c = np.empty((E_LOCAL, FK, 128, D_MODEL), dtype=ml_dtypes.bfloat16)
        for e in range(E_LOCAL):
            g = int(slot_expert[c, e])
            L = Ls[e]
            toks = tok_sorted[seg[g]:seg[g + 1]]
            wts = w_sorted[seg[g]:seg[g + 1]]
            xg = np.zeros((D_MODEL, L), dtype=ml_dtypes.bfloat16)
            xv = np.zeros((D_MODEL, L), dtype=ml_dtypes.bfloat16)
            xg[:, :len(toks)] = xb[toks].T
            xv[:, :len(toks)] = _to_bf16(x[toks] * wts[:, None]).T
            m[f"xg{e}"] = xg.reshape(DK, 128, L)
            m[f"xv{e}"] = xv.reshape(DK, 128, L)
            w1_c[e] = w1b[g].reshape(DK, 128, D_FF)
            w2_c[e] = w2b[g].reshape(DK, 128, D_FF)
            w3_c[e] = w3b[g].reshape(FK, 128, D_MODEL)
        m["w1t"] = w1_c
        m["w2t"] = w2_c
        m["w3t"] = w3_c
        in_maps.append(m)

    nc = _get_nc(L0, L1)
    meta = {"seg": seg, "tok_sorted": tok_sorted, "Ls": Ls, "slot_expert": slot_expert}
    return nc, in_maps, meta


def combine(results, meta):
    """Unshard per-core expert outputs and sum the top-2 contributions."""
    seg, tok_sorted, Ls = meta["seg"], meta["tok_sorted"], meta["Ls"]
    slot_expert = meta["slot_expert"]
    assign_rows = np.empty((N_TOKENS * TOP_K, D_MODEL), dtype=np.float32)
    for c in range(N_CORES):
        for e in range(E_LOCAL):
            g = int(slot_expert[c, e])
            cnt = seg[g + 1] - seg[g]
            ytc = results[c][f"yt{e}"].reshape(D_MODEL, Ls[e])
            assign_rows[seg[g]:seg[g + 1]] = ytc[:, :cnt].T

    by_token = np.argsort(tok_sorted, kind="stable")
    out = assign_rows[by_token].reshape(N_TOKENS, TOP_K, D_MODEL).sum(axis=1)
    return out.astype(np.float32)


def kernel(x, expert_indices, expert_weights, w1, w2, w3, _run_opts=None):
    from concourse.bass_utils import run_bass_kernel_spmd

    nc, in_maps, meta = prepare(x, expert_indices, expert_weights, w1, w2, w3)
    opts = dict(_run_opts or {})
    res = run_bass_kernel_spmd(nc, in_maps, list(range(N_CORES)), **opts)
    if _run_opts is not None:
        _run_opts["result"] = res
    return combine(res.results, meta)
